# revision 16
# baseline (speedup 1.0000x reference)
"""Trainium2 Bass kernel for nn_C4ByteTransformer (4-step carry-propagation
softmax table lookup).

Contract: kernel(**inputs) takes FULL inputs (a_emb[4,256], b_emb[4,256],
W1[514,131072], W2_sum[131072,256], W2_carry[131072,2]) and returns the full
[4,256] float32 output.

Math: the tables are the canonical one-hot construction (verified exactly on
host, numpy fallback otherwise), so
  scores[k] = a_i[a(k)] + b_i[b(k)] + carry[c(k)],  k = 512a + 2b + c.
The softmax weights therefore factorize rank-1 per carry slice:
  w[a,b,c] ~ alpha[a] * beta[b] * gamma_c,
  alpha = exp(10*a_i), beta = exp(10*b_i - 25), gamma_c = exp(10*carry_c),
and the entire step reduces to a length-256 circular convolution
  T[m]   = sum_a alpha[a] * beta[(m-a) & 255]          (result numerator)
plus two triangular sums for the carry:
  U1     = sum_{a+b>=256} alpha[a]*beta[b]
  U1'    = sum_{a+b>=255} alpha[a]*beta[b]
  Z      = sum(alpha) * sum(beta)
With p = P(carry=1), lam = sigmoid(10*(2p-1)):
  out_s  = (T + lam*(rot1(T) - T)) / Z
  p_next = (U1 + lam*(U1' - U1)) / Z
No table reads, no collectives: each core runs the identical tiny program
(~1.5MB of host-gathered circulant/mask operands) and core 0's output is
returned.  The convolution is one 8-chunk f32 matmul against a host-gathered
circulant of b_emb; the triangular sums are one 2-chunk matmul against
constant 0/1 masks; the 4-step carry chain runs on [1,1] scalars.
"""

import os

import numpy as np

N_CORES = 8
D = 256
NSTEP = 4
NE = 256 * 256 * 2
SCALE = 10.0
BIAS_B = -25.0

_CACHE = {}

LAST_EXEC_TIME_NS = None


def _build_nc():
    import concourse.bacc as bacc
    import concourse.mybir as mybir
    import concourse.tile as tile

    f32 = mybir.dt.float32
    mult = mybir.AluOpType.mult
    add = mybir.AluOpType.add
    subtract = mybir.AluOpType.subtract
    Exp = mybir.ActivationFunctionType.Exp
    Sigmoid = mybir.ActivationFunctionType.Sigmoid

    nc = bacc.Bacc("TRN2", target_bir_lowering=False, debug=False,
                   num_devices=N_CORES)

    # Inputs (host pre-gathered; identical on every core).
    cb = nc.dram_tensor("cb", [128, 8, D], f32, kind="ExternalInput")
    m12 = nc.dram_tensor("m12", [128, 2, 2 * D], f32, kind="ExternalInput")
    atr = nc.dram_tensor("atr", [128, 8], f32, kind="ExternalInput")
    arm = nc.dram_tensor("arm", [128, 2, NSTEP], f32, kind="ExternalInput")
    bp = nc.dram_tensor("bp", [NSTEP, D], f32, kind="ExternalInput")
    ap_in = nc.dram_tensor("ap_in", [NSTEP, D], f32, kind="ExternalInput")
    out = nc.dram_tensor("out", [NSTEP, D], f32, kind="ExternalOutput")
    q_dram = nc.dram_tensor("q_dram", [NSTEP, 2], f32)
    lam_dram = nc.dram_tensor("lam_dram", [NSTEP, 1], f32)

    with tile.TileContext(nc) as tc:
        with (
            tc.tile_pool(name="big", bufs=1) as big,
            tc.tile_pool(name="small", bufs=1) as small,
            tc.tile_pool(name="ps", bufs=1, space="PSUM") as ps,
        ):
            # ---- DMAs: big circulant split across the two HWDGE queues ----
            cb_sb = big.tile([128, 8, D], f32)
            nc.sync.dma_start(cb_sb[:, 0:4, :], cb[:, 0:4, :])
            atr_sb = small.tile([128, 8], f32)
            nc.scalar.dma_start(atr_sb[:], atr[:])
            arm_sb = small.tile([128, 2, NSTEP], f32)
            nc.scalar.dma_start(arm_sb[:], arm[:])
            bp_sb = small.tile([NSTEP, D], f32)
            nc.scalar.dma_start(bp_sb[:], bp[:])
            ap_sb = small.tile([NSTEP, D], f32)
            nc.scalar.dma_start(ap_sb[:], ap_in[:])
            nc.scalar.dma_start(cb_sb[:, 4:8, :], cb[:, 4:8, :])
            m12_sb = big.tile([128, 2, 2 * D], f32)
            nc.sync.dma_start(m12_sb[:], m12[:])

            # Constant bias tiles for ACT (float biases need const APs).
            cst128 = small.tile([128, 2], f32)
            nc.vector.memset(cst128[:, 0:1], 0.0)
            nc.vector.memset(cst128[:, 1:2], BIAS_B)
            cst4 = small.tile([NSTEP, 2], f32)
            nc.vector.memset(cst4[:, 0:1], 0.0)
            nc.vector.memset(cst4[:, 1:2], BIAS_B)
            cst1 = small.tile([1, 1], f32)
            nc.vector.memset(cst1[:], -10.0)

            # ---- Exponentials ----
            # Block-diagonal conv lhsT: at_t[:, 2s:2s+2, s] = exp(10*atr)
            at_t = small.tile([128, 8, NSTEP], f32)
            nc.vector.memset(at_t[:], 0.0)
            for s in range(NSTEP):
                nc.scalar.activation(
                    at_t[:, 2 * s : 2 * s + 2, s], atr_sb[:, 2 * s : 2 * s + 2],
                    Exp, bias=cst128[:, 0:1], scale=SCALE,
                )
            arm_e = small.tile([128, 2, NSTEP], f32)
            nc.scalar.activation(arm_e[:], arm_sb[:], Exp, bias=cst128[:, 0:1],
                                 scale=SCALE)
            cbe = big.tile([128, 8, D], f32)
            nc.scalar.activation(cbe[:, 0:4, :], cb_sb[:, 0:4, :], Exp,
                                 bias=cst128[:, 1:2], scale=SCALE)
            nc.scalar.activation(cbe[:, 4:8, :], cb_sb[:, 4:8, :], Exp,
                                 bias=cst128[:, 1:2], scale=SCALE)
            sums = small.tile([NSTEP, 2], f32)  # (sum alpha, sum beta)
            ape = small.tile([NSTEP, D], f32)
            nc.scalar.activation(ape[:], ap_sb[:], Exp, bias=cst4[:, 0:1],
                                 scale=SCALE)
            bpe = small.tile([NSTEP, D], f32)
            nc.scalar.activation(bpe[:], bp_sb[:], Exp, bias=cst4[:, 1:2],
                                 scale=SCALE)
            nc.vector.tensor_reduce(out=sums[:, 0:1], in_=ape[:],
                                    axis=mybir.AxisListType.X,
                                    op=mybir.AluOpType.add)
            nc.vector.tensor_reduce(out=sums[:, 1:2], in_=bpe[:],
                                    axis=mybir.AxisListType.X,
                                    op=mybir.AluOpType.add)

            # ---- Matmuls: conv T [4,256]; masks -> (w1|w2) [4,512] ----
            pc = ps.tile([NSTEP, D], f32, tag="pc")
            for c in range(8):
                nc.tensor.matmul(pc[:], lhsT=at_t[:, c, :], rhs=cbe[:, c, :],
                                 start=(c == 0), stop=(c == 7))
            pm = ps.tile([NSTEP, 2 * D], f32, tag="pm")
            for h in range(2):
                nc.tensor.matmul(pm[:], lhsT=arm_e[:, h, :], rhs=m12_sb[:, h, :],
                                 start=(h == 0), stop=(h == 1))

            # ---- Scalars: Z, U1, U1' -> q1, q2 ----
            zz = small.tile([NSTEP, 2], f32)  # (Z, 1/Z)
            nc.vector.tensor_tensor(out=zz[:, 0:1], in0=sums[:, 0:1],
                                    in1=sums[:, 1:2], op=mult)
            nc.vector.reciprocal(zz[:, 1:2], zz[:, 0:1])
            uu = small.tile([NSTEP, 2], f32)  # (U1, U1')
            scr = small.tile([NSTEP, D], f32)
            nc.vector.tensor_tensor(out=scr[:], in0=pm[:, 0:D], in1=bpe[:],
                                    op=mult)
            nc.vector.tensor_reduce(out=uu[:, 0:1], in_=scr[:],
                                    axis=mybir.AxisListType.X,
                                    op=mybir.AluOpType.add)
            nc.vector.tensor_tensor(out=scr[:], in0=pm[:, D : 2 * D],
                                    in1=bpe[:], op=mult)
            nc.vector.tensor_reduce(out=uu[:, 1:2], in_=scr[:],
                                    axis=mybir.AxisListType.X,
                                    op=mybir.AluOpType.add)
            chn = small.tile([NSTEP, 2], f32)  # (q1, q2) = (U1/Z, (U1'-U1)/Z)
            nc.vector.tensor_tensor(out=chn[:, 0:1], in0=uu[:, 0:1],
                                    in1=zz[:, 1:2], op=mult)
            nc.vector.tensor_tensor(out=chn[:, 1:2], in0=uu[:, 1:2],
                                    in1=uu[:, 0:1], op=subtract)
            nc.vector.tensor_tensor(out=chn[:, 1:2], in0=chn[:, 1:2],
                                    in1=zz[:, 1:2], op=mult)

            # ---- Repartition q1, q2 to partition-0 rows via DRAM bounce ----
            nc.sync.dma_start(q_dram[:], chn[:])
            ptc1 = small.tile([1, NSTEP], f32)
            nc.sync.dma_start(ptc1[:], q_dram[:, 0:1])
            ptc2 = small.tile([1, NSTEP], f32)
            nc.scalar.dma_start(ptc2[:], q_dram[:, 1:2])

            # ---- Sequential carry chain on [1,1] scalars ----
            pcur = small.tile([1, 1], f32)
            nc.vector.memset(pcur[:], 0.0)
            lam = small.tile([1, NSTEP], f32)
            for s in range(NSTEP):
                nc.scalar.activation(lam[:, s : s + 1], pcur[:], Sigmoid,
                                     bias=cst1[:], scale=20.0)
                if s + 1 < NSTEP:
                    nc.vector.scalar_tensor_tensor(
                        out=pcur[:], in0=lam[:, s : s + 1],
                        scalar=ptc2[:, s : s + 1], in1=ptc1[:, s : s + 1],
                        op0=mult, op1=add,
                    )

            # ---- lam back to a [4,1] column via DRAM bounce ----
            nc.sync.dma_start(lam_dram[:], lam[:])
            lamc = small.tile([NSTEP, 1], f32)
            nc.sync.dma_start(lamc[:], lam_dram[:])

            # ---- Final blend: out = (T + lam*(rot1(T)-T)) / Z ----
            tsb = small.tile([NSTEP, D], f32)
            nc.vector.tensor_copy(out=tsb[:], in_=pc[:])
            trot = small.tile([NSTEP, D], f32)
            nc.vector.tensor_copy(out=trot[:, 1:D], in_=tsb[:, 0 : D - 1])
            nc.vector.tensor_copy(out=trot[:, 0:1], in_=tsb[:, D - 1 : D])
            nc.vector.tensor_tensor(out=trot[:], in0=trot[:], in1=tsb[:],
                                    op=subtract)
            res = small.tile([NSTEP, D], f32)
            nc.vector.tensor_scalar(out=res[:], in0=trot[:],
                                    scalar1=lamc[:, 0:1], scalar2=None,
                                    op0=mult)
            nc.vector.tensor_tensor(out=res[:], in0=res[:], in1=tsb[:], op=add)
            nc.vector.tensor_scalar(out=res[:], in0=res[:],
                                    scalar1=zz[:, 1:2], scalar2=None, op0=mult)
            nc.sync.dma_start(out[:], res[:])

    nc.compile()
    return nc


def _structure_ok(W1, W2_sum, W2_carry):
    """Exact check that the tables are the canonical one-hot construction."""
    if W1.shape != (514, NE) or W2_sum.shape != (NE, D) or W2_carry.shape != (NE, 2):
        return False
    k = np.arange(NE)
    a = k >> 9
    b = (k >> 1) & 255
    c = k & 1
    tot = a + b + c
    if not (W1[a, k] == 1.0).all():
        return False
    if not (W1[D + b, k] == 1.0).all():
        return False
    if not (W1[2 * D + c, k] == 1.0).all():
        return False
    if W1.sum(dtype=np.float64) != 3.0 * NE or W1.min() < 0.0:
        return False
    if not (W2_sum[k, tot & 255] == 1.0).all():
        return False
    if W2_sum.sum(dtype=np.float64) != NE or W2_sum.min() < 0.0:
        return False
    if not (W2_carry[k, (tot >= 256).astype(np.int64)] == 1.0).all():
        return False
    if W2_carry.sum(dtype=np.float64) != NE or W2_carry.min() < 0.0:
        return False
    return True


def _numpy_fallback(a_emb, b_emb, W1, W2_sum, W2_carry):
    carry = np.zeros(2, dtype=np.float64)
    carry[0] = 1.0
    outs = []
    W1 = W1.astype(np.float64)
    for i in range(NSTEP):
        x = np.concatenate([a_emb[i], b_emb[i], carry]).astype(np.float64)
        scores = x @ W1
        z = (scores - 2.5) * 10.0
        z -= z.max()
        w = np.exp(z)
        w /= w.sum()
        outs.append(w @ W2_sum.astype(np.float64))
        carry = w @ W2_carry.astype(np.float64)
    return np.stack(outs).astype(np.float32)


def _prep_inputs(a_emb, b_emb):
    """Pure-layout host gathers; every core gets the identical map."""
    idx = np.arange(D)
    # cb[p, 2s+h, m] = b_emb[s, (m + 128h + p) & 255]
    p = np.arange(128)
    cb = np.empty((128, 8, D), np.float32)
    atr = np.empty((128, 8), np.float32)
    arm = np.empty((128, 2, NSTEP), np.float32)
    for s in range(NSTEP):
        for h in range(2):
            aprime = 128 * h + p  # [128]
            cb[:, 2 * s + h, :] = b_emb[s][(idx[None, :] + aprime[:, None]) & 255]
            vals = a_emb[s][(256 - aprime) & 255]
            atr[:, 2 * s + h] = vals
            arm[:, h, s] = vals

    # m12[p, h, j] = M1[128h+p, j]; m12[p, h, 256+j] = M2[128h+p, j]
    aprime = idx[:, None]  # [256,1]
    M1 = ((idx[None, :] >= aprime) & (aprime >= 1)).astype(np.float32)
    M2 = ((idx[None, :] >= aprime - 1) & (aprime >= 1)).astype(np.float32)
    M2[0, :] = 0.0
    M2[0, 255] = 1.0
    m12 = np.empty((128, 2, 2 * D), np.float32)
    for h in range(2):
        m12[:, h, 0:D] = M1[128 * h : 128 * h + 128, :]
        m12[:, h, D : 2 * D] = M2[128 * h : 128 * h + 128, :]

    one = {
        "cb": cb,
        "m12": m12,
        "atr": atr,
        "arm": arm,
        "bp": np.ascontiguousarray(b_emb, dtype=np.float32),
        "ap_in": np.ascontiguousarray(a_emb, dtype=np.float32),
    }
    return [dict(one) for _ in range(N_CORES)]


def kernel(a_emb, b_emb, W1, W2_sum, W2_carry):
    global LAST_EXEC_TIME_NS
    a_emb = np.asarray(a_emb, dtype=np.float32)
    b_emb = np.asarray(b_emb, dtype=np.float32)
    W1 = np.asarray(W1, dtype=np.float32)
    W2_sum = np.asarray(W2_sum, dtype=np.float32)
    W2_carry = np.asarray(W2_carry, dtype=np.float32)

    if not _structure_ok(W1, W2_sum, W2_carry):
        return _numpy_fallback(a_emb, b_emb, W1, W2_sum, W2_carry)

    from concourse.bass_utils import run_bass_kernel_spmd

    if "nc" not in _CACHE:
        _CACHE["nc"] = _build_nc()
    nc = _CACHE["nc"]

    in_maps = _prep_inputs(a_emb, b_emb)
    trace = os.environ.get("KERNEL_TRACE", "") == "1"
    res = run_bass_kernel_spmd(nc, in_maps, list(range(N_CORES)), trace=trace)
    LAST_EXEC_TIME_NS = res.exec_time_ns
    return np.asarray(res.results[0]["out"], dtype=np.float32)


# revision 24
# speedup vs baseline: 1.1476x; 1.1476x over previous
"""Trainium2 Bass kernel for nn_C4ByteTransformer (4-step carry-propagation
softmax table lookup).

Contract: kernel(**inputs) takes FULL inputs (a_emb[4,256], b_emb[4,256],
W1[514,131072], W2_sum[131072,256], W2_carry[131072,2]) and returns the full
[4,256] float32 output.

Math: the tables are the canonical one-hot construction (verified exactly on
host, numpy fallback otherwise), so
  scores[k] = a_i[a(k)] + b_i[b(k)] + carry[c(k)],  k = 512a + 2b + c.
The softmax weights therefore factorize rank-1 per carry slice:
  w[a,b,c] ~ alpha[a] * beta[b] * gamma_c,
  alpha = exp(10*a_i), beta = exp(10*b_i - 25), gamma_c = exp(10*carry_c),
and the entire step reduces to a length-256 circular convolution
  T[m]   = sum_a alpha[a] * beta[(m-a) & 255]          (result numerator)
plus two triangular sums for the carry:
  U1     = sum_{a+b>=256} alpha[a]*beta[b]
  U1'    = sum_{a+b>=255} alpha[a]*beta[b]
  Z      = sum(alpha) * sum(beta)
With p = P(carry=1), lam = sigmoid(10*(2p-1)):
  out_s  = (T + lam*(rot1(T) - T)) / Z
  p_next = (U1 + lam*(U1' - U1)) / Z
No table reads, no collectives: each core runs the identical tiny program
(~1.5MB of host-gathered circulant/mask operands) and core 0's output is
returned.  The convolution is one 8-chunk f32 matmul against a host-gathered
circulant of b_emb; the triangular sums are one 2-chunk matmul against
constant 0/1 masks; the 4-step carry chain runs on [1,1] scalars.
"""

import os

import numpy as np

N_CORES = 8
D = 256
NSTEP = 4
NE = 256 * 256 * 2
SCALE = 10.0
BIAS_B = -25.0

_CACHE = {}

LAST_EXEC_TIME_NS = None


def _build_nc():
    import concourse.bacc as bacc
    import concourse.mybir as mybir
    import concourse.tile as tile

    f32 = mybir.dt.float32
    mult = mybir.AluOpType.mult
    add = mybir.AluOpType.add
    subtract = mybir.AluOpType.subtract
    Exp = mybir.ActivationFunctionType.Exp
    Sigmoid = mybir.ActivationFunctionType.Sigmoid

    nc = bacc.Bacc("TRN2", target_bir_lowering=False, debug=False,
                   num_devices=N_CORES)

    # Inputs (host pre-gathered; identical on every core).
    cb = nc.dram_tensor("cb", [128, 8, D], f32, kind="ExternalInput")
    m12 = nc.dram_tensor("m12", [128, 2, 2 * D], f32, kind="ExternalInput")
    atrarm = nc.dram_tensor("atrarm", [128, 16], f32, kind="ExternalInput")
    bpap = nc.dram_tensor("bpap", [NSTEP, 2 * D], f32, kind="ExternalInput")
    out = nc.dram_tensor("out", [NSTEP, D], f32, kind="ExternalOutput")

    with tile.TileContext(nc) as tc:
        with (
            tc.tile_pool(name="big", bufs=1) as big,
            tc.tile_pool(name="small", bufs=1) as small,
            tc.tile_pool(name="ps", bufs=1, space="PSUM") as ps,
        ):
            # ---- DMAs: packed smalls first, big circulant split across the
            # two HWDGE queues, masks last (needed ~mid-kernel) ----
            atrarm_sb = small.tile([128, 16], f32)
            nc.sync.dma_start(atrarm_sb[:], atrarm[:])
            bpap_sb = small.tile([NSTEP, 2 * D], f32)
            nc.scalar.dma_start(bpap_sb[:], bpap[:])
            cb_sb = big.tile([128, 8, D], f32)
            nc.sync.dma_start(cb_sb[:, 0:4, :], cb[:, 0:4, :])
            nc.scalar.dma_start(cb_sb[:, 4:8, :], cb[:, 4:8, :])
            m12_sb = big.tile([128, 2, 2 * D], f32)
            nc.sync.dma_start(m12_sb[:], m12[:])

            # Constant bias tiles for ACT (float biases need const APs).
            cst128 = small.tile([128, 2], f32)
            nc.vector.memset(cst128[:, 0:1], 0.0)
            nc.vector.memset(cst128[:, 1:2], BIAS_B)
            cst4 = small.tile([NSTEP, 2], f32)
            nc.vector.memset(cst4[:, 0:1], 0.0)
            nc.vector.memset(cst4[:, 1:2], BIAS_B)
            cst1 = small.tile([1, 1], f32)
            nc.vector.memset(cst1[:], -10.0)

            # ---- Exponentials (mask-matmul operands first) ----
            arm_e = small.tile([128, 8], f32)  # [p, 4h+s] = alphar_s chunk h
            nc.scalar.activation(arm_e[:], atrarm_sb[:, 8:16], Exp,
                                 bias=cst128[:, 0:1], scale=SCALE)
            # Block-diagonal conv lhsT: at_t[:, 2s:2s+2, s] = exp(10*atr)
            at_t = small.tile([128, 8, NSTEP], f32)
            nc.vector.memset(at_t[:], 0.0)
            for s in range(NSTEP):
                nc.scalar.activation(
                    at_t[:, 2 * s : 2 * s + 2, s],
                    atrarm_sb[:, 2 * s : 2 * s + 2],
                    Exp, bias=cst128[:, 0:1], scale=SCALE,
                )
            sums = small.tile([NSTEP, 2], f32)  # (sum alpha, sum beta)
            bpe = small.tile([NSTEP, D], f32)
            nc.scalar.activation(bpe[:], bpap_sb[:, 0:D], Exp,
                                 bias=cst4[:, 1:2], scale=SCALE)
            ape = small.tile([NSTEP, D], f32)
            nc.scalar.activation(ape[:], bpap_sb[:, D : 2 * D], Exp,
                                 bias=cst4[:, 0:1], scale=SCALE)
            cbe = big.tile([128, 8, D], f32)
            nc.scalar.activation(cbe[:, 0:4, :], cb_sb[:, 0:4, :], Exp,
                                 bias=cst128[:, 1:2], scale=SCALE)
            nc.scalar.activation(cbe[:, 4:8, :], cb_sb[:, 4:8, :], Exp,
                                 bias=cst128[:, 1:2], scale=SCALE)
            nc.vector.tensor_reduce(out=sums[:, 0:1], in_=ape[:],
                                    axis=mybir.AxisListType.X,
                                    op=mybir.AluOpType.add)
            nc.vector.tensor_reduce(out=sums[:, 1:2], in_=bpe[:],
                                    axis=mybir.AxisListType.X,
                                    op=mybir.AluOpType.add)

            # ---- Matmuls: masks first (they head the long scalar-chain
            # dependency path), conv T after ----
            pm = ps.tile([NSTEP, 2 * D], f32, tag="pm")
            for h in range(2):
                nc.tensor.matmul(pm[:], lhsT=arm_e[:, 4 * h : 4 * h + 4],
                                 rhs=m12_sb[:, h, :],
                                 start=(h == 0), stop=(h == 1))
            pc = ps.tile([NSTEP, D], f32, tag="pc")
            for c in range(8):
                nc.tensor.matmul(pc[:], lhsT=at_t[:, c, :], rhs=cbe[:, c, :],
                                 start=(c == 0), stop=(c == 7))

            # ---- Scalars: Z, U1, U1' -> q1, q2 ----
            zz = small.tile([NSTEP, 2], f32)  # (Z, 1/Z)
            nc.vector.tensor_tensor(out=zz[:, 0:1], in0=sums[:, 0:1],
                                    in1=sums[:, 1:2], op=mult)
            nc.vector.reciprocal(zz[:, 1:2], zz[:, 0:1])
            uu = small.tile([NSTEP, 2], f32)  # (U1, U1')
            scr = small.tile([NSTEP, D], f32)
            nc.vector.tensor_tensor(out=scr[:], in0=pm[:, 0:D], in1=bpe[:],
                                    op=mult)
            nc.vector.tensor_reduce(out=uu[:, 0:1], in_=scr[:],
                                    axis=mybir.AxisListType.X,
                                    op=mybir.AluOpType.add)
            nc.vector.tensor_tensor(out=scr[:], in0=pm[:, D : 2 * D],
                                    in1=bpe[:], op=mult)
            nc.vector.tensor_reduce(out=uu[:, 1:2], in_=scr[:],
                                    axis=mybir.AxisListType.X,
                                    op=mybir.AluOpType.add)
            chn = small.tile([NSTEP, 2], f32)  # (q1, q2) = (U1/Z, (U1'-U1)/Z)
            nc.vector.tensor_tensor(out=chn[:, 0:1], in0=uu[:, 0:1],
                                    in1=zz[:, 1:2], op=mult)
            nc.vector.tensor_tensor(out=chn[:, 1:2], in0=uu[:, 1:2],
                                    in1=uu[:, 0:1], op=subtract)
            nc.vector.tensor_tensor(out=chn[:, 1:2], in0=chn[:, 1:2],
                                    in1=zz[:, 1:2], op=mult)

            # ---- Repartition (q1,q2) to a partition-0 row: SBUF->SBUF DMA ----
            ptc12 = small.tile([1, NSTEP, 2], f32)
            nc.sync.dma_start(ptc12[:], chn[:])

            # ---- Sequential carry chain on [1,1] scalars ----
            pcur = small.tile([1, 1], f32)
            nc.vector.memset(pcur[:], 0.0)
            lam = small.tile([1, NSTEP], f32)
            for s in range(NSTEP):
                nc.scalar.activation(lam[:, s : s + 1], pcur[:], Sigmoid,
                                     bias=cst1[:], scale=20.0)
                if s + 1 < NSTEP:
                    nc.vector.scalar_tensor_tensor(
                        out=pcur[:], in0=lam[:, s : s + 1],
                        scalar=ptc12[:, s, 1:2], in1=ptc12[:, s, 0:1],
                        op0=mult, op1=add,
                    )

            # ---- lam back to a [4,1] column: SBUF->SBUF DMA ----
            lamc = small.tile([NSTEP, 1], f32)
            nc.sync.dma_start(lamc[:], lam[:])

            # ---- Final blend: out = (T + lam*(rot1(T)-T)) / Z ----
            tsb = small.tile([NSTEP, D], f32)
            nc.vector.tensor_copy(out=tsb[:], in_=pc[:])
            trot = small.tile([NSTEP, D], f32)
            nc.vector.tensor_copy(out=trot[:, 1:D], in_=tsb[:, 0 : D - 1])
            nc.vector.tensor_copy(out=trot[:, 0:1], in_=tsb[:, D - 1 : D])
            nc.vector.tensor_tensor(out=trot[:], in0=trot[:], in1=tsb[:],
                                    op=subtract)
            res = small.tile([NSTEP, D], f32)
            nc.vector.tensor_scalar(out=res[:], in0=trot[:],
                                    scalar1=lamc[:, 0:1], scalar2=None,
                                    op0=mult)
            nc.vector.tensor_tensor(out=res[:], in0=res[:], in1=tsb[:], op=add)
            nc.vector.tensor_scalar(out=res[:], in0=res[:],
                                    scalar1=zz[:, 1:2], scalar2=None, op0=mult)
            nc.sync.dma_start(out[:], res[:])

    nc.compile()
    return nc


def _structure_ok(W1, W2_sum, W2_carry):
    """Exact check that the tables are the canonical one-hot construction."""
    if W1.shape != (514, NE) or W2_sum.shape != (NE, D) or W2_carry.shape != (NE, 2):
        return False
    k = np.arange(NE)
    a = k >> 9
    b = (k >> 1) & 255
    c = k & 1
    tot = a + b + c
    if not (W1[a, k] == 1.0).all():
        return False
    if not (W1[D + b, k] == 1.0).all():
        return False
    if not (W1[2 * D + c, k] == 1.0).all():
        return False
    if W1.sum(dtype=np.float64) != 3.0 * NE or W1.min() < 0.0:
        return False
    if not (W2_sum[k, tot & 255] == 1.0).all():
        return False
    if W2_sum.sum(dtype=np.float64) != NE or W2_sum.min() < 0.0:
        return False
    if not (W2_carry[k, (tot >= 256).astype(np.int64)] == 1.0).all():
        return False
    if W2_carry.sum(dtype=np.float64) != NE or W2_carry.min() < 0.0:
        return False
    return True


def _numpy_fallback(a_emb, b_emb, W1, W2_sum, W2_carry):
    carry = np.zeros(2, dtype=np.float64)
    carry[0] = 1.0
    outs = []
    W1 = W1.astype(np.float64)
    for i in range(NSTEP):
        x = np.concatenate([a_emb[i], b_emb[i], carry]).astype(np.float64)
        scores = x @ W1
        z = (scores - 2.5) * 10.0
        z -= z.max()
        w = np.exp(z)
        w /= w.sum()
        outs.append(w @ W2_sum.astype(np.float64))
        carry = w @ W2_carry.astype(np.float64)
    return np.stack(outs).astype(np.float32)


def _prep_inputs(a_emb, b_emb):
    """Pure-layout host gathers; every core gets the identical map."""
    idx = np.arange(D)
    # cb[p, 2s+h, m] = b_emb[s, (m + 128h + p) & 255]
    p = np.arange(128)
    cb = np.empty((128, 8, D), np.float32)
    atrarm = np.empty((128, 16), np.float32)
    for s in range(NSTEP):
        for h in range(2):
            aprime = 128 * h + p  # [128]
            cb[:, 2 * s + h, :] = b_emb[s][(idx[None, :] + aprime[:, None]) & 255]
            vals = a_emb[s][(256 - aprime) & 255]
            atrarm[:, 2 * s + h] = vals
            atrarm[:, 8 + 4 * h + s] = vals

    # m12[p, h, j] = M1[128h+p, j]; m12[p, h, 256+j] = M2[128h+p, j]
    aprime = idx[:, None]  # [256,1]
    M1 = ((idx[None, :] >= aprime) & (aprime >= 1)).astype(np.float32)
    M2 = ((idx[None, :] >= aprime - 1) & (aprime >= 1)).astype(np.float32)
    M2[0, :] = 0.0
    M2[0, 255] = 1.0
    m12 = np.empty((128, 2, 2 * D), np.float32)
    for h in range(2):
        m12[:, h, 0:D] = M1[128 * h : 128 * h + 128, :]
        m12[:, h, D : 2 * D] = M2[128 * h : 128 * h + 128, :]

    bpap = np.empty((NSTEP, 2 * D), np.float32)
    bpap[:, 0:D] = b_emb
    bpap[:, D : 2 * D] = a_emb
    one = {
        "cb": cb,
        "m12": m12,
        "atrarm": atrarm,
        "bpap": bpap,
    }
    return [dict(one) for _ in range(N_CORES)]


def kernel(a_emb, b_emb, W1, W2_sum, W2_carry):
    global LAST_EXEC_TIME_NS
    a_emb = np.asarray(a_emb, dtype=np.float32)
    b_emb = np.asarray(b_emb, dtype=np.float32)
    W1 = np.asarray(W1, dtype=np.float32)
    W2_sum = np.asarray(W2_sum, dtype=np.float32)
    W2_carry = np.asarray(W2_carry, dtype=np.float32)

    if not _structure_ok(W1, W2_sum, W2_carry):
        return _numpy_fallback(a_emb, b_emb, W1, W2_sum, W2_carry)

    from concourse.bass_utils import run_bass_kernel_spmd

    if "nc" not in _CACHE:
        _CACHE["nc"] = _build_nc()
    nc = _CACHE["nc"]

    in_maps = _prep_inputs(a_emb, b_emb)
    trace = os.environ.get("KERNEL_TRACE", "") == "1"
    res = run_bass_kernel_spmd(nc, in_maps, list(range(N_CORES)), trace=trace)
    LAST_EXEC_TIME_NS = res.exec_time_ns
    return np.asarray(res.results[0]["out"], dtype=np.float32)


# revision 29
# speedup vs baseline: 1.2411x; 1.0815x over previous
"""Trainium2 Bass kernel for nn_C4ByteTransformer (4-step carry-propagation
softmax table lookup).

Contract: kernel(**inputs) takes FULL inputs (a_emb[4,256], b_emb[4,256],
W1[514,131072], W2_sum[131072,256], W2_carry[131072,2]) and returns the full
[4,256] float32 output.

Math: the tables are the canonical one-hot construction (verified exactly on
host, numpy fallback otherwise), so
  scores[k] = a_i[a(k)] + b_i[b(k)] + carry[c(k)],  k = 512a + 2b + c.
The softmax weights therefore factorize rank-1 per carry slice:
  w[a,b,c] ~ alpha[a] * beta[b] * gamma_c,
  alpha = exp(10*a_i), beta = exp(10*b_i - 25), gamma_c = exp(10*carry_c),
and the entire step reduces to a length-256 circular convolution
  T[m]   = sum_a alpha[a] * beta[(m-a) & 255]          (result numerator)
plus two triangular sums for the carry:
  U1     = sum_{a+b>=256} alpha[a]*beta[b]
  U1'    = sum_{a+b>=255} alpha[a]*beta[b]
  Z      = sum(alpha) * sum(beta)
With p = P(carry=1), lam = sigmoid(10*(2p-1)):
  out_s  = (T + lam*(rot1(T) - T)) / Z
  p_next = (U1 + lam*(U1' - U1)) / Z
No table reads, no collectives: each core runs the identical tiny program
(~1.5MB of host-gathered circulant/mask operands) and core 0's output is
returned.  The convolution is one 8-chunk f32 matmul against a host-gathered
circulant of b_emb; the triangular sums are one 2-chunk matmul against
constant 0/1 masks; the 4-step carry chain runs on [1,1] scalars.
"""

import os

import numpy as np

N_CORES = 8
D = 256
NSTEP = 4
NE = 256 * 256 * 2
SCALE = 10.0
BIAS_B = -25.0

_CACHE = {}

LAST_EXEC_TIME_NS = None


def _build_nc():
    import concourse.bacc as bacc
    import concourse.mybir as mybir
    import concourse.tile as tile

    f32 = mybir.dt.float32
    f32r = mybir.dt.float32r
    mult = mybir.AluOpType.mult
    add = mybir.AluOpType.add
    subtract = mybir.AluOpType.subtract
    Exp = mybir.ActivationFunctionType.Exp
    Sigmoid = mybir.ActivationFunctionType.Sigmoid

    nc = bacc.Bacc("TRN2", target_bir_lowering=False, debug=False,
                   num_devices=N_CORES)

    # Inputs (host pre-gathered; identical on every core).
    cb = nc.dram_tensor("cb", [128, 8, D], f32, kind="ExternalInput")
    m12 = nc.dram_tensor("m12", [128, 2, 2 * D], f32r, kind="ExternalInput")
    atd = nc.dram_tensor("atd", [128, 8, NSTEP], f32, kind="ExternalInput")
    armn = nc.dram_tensor("armn", [128, 2, NSTEP], f32, kind="ExternalInput")
    bpap = nc.dram_tensor("bpap", [NSTEP, 2 * D], f32, kind="ExternalInput")
    out = nc.dram_tensor("out", [NSTEP, D], f32, kind="ExternalOutput")

    with tile.TileContext(nc) as tc:
        with (
            tc.tile_pool(name="big", bufs=1) as big,
            tc.tile_pool(name="small", bufs=1) as small,
            tc.tile_pool(name="ps", bufs=1, space="PSUM") as ps,
        ):
            # ---- DMAs: packed smalls first, big circulant split across the
            # two HWDGE queues, masks last (needed ~mid-kernel) ----
            atd_sb = small.tile([128, 8, NSTEP], f32)
            nc.sync.dma_start(atd_sb[:], atd[:])
            armn_sb = small.tile([128, 2, NSTEP], f32)
            nc.scalar.dma_start(armn_sb[:], armn[:])
            bpap_sb = small.tile([NSTEP, 2 * D], f32)
            nc.scalar.dma_start(bpap_sb[:], bpap[:])
            m12_sb = big.tile([128, 2, 2 * D], f32r)
            nc.sync.dma_start(m12_sb[:], m12[:])
            cb_sb = big.tile([128, 8, D], f32)
            nc.sync.dma_start(cb_sb[:, 0:4, :], cb[:, 0:4, :])
            nc.scalar.dma_start(cb_sb[:, 4:8, :], cb[:, 4:8, :])

            # Constant bias tiles for ACT (float biases need const APs).
            cst128 = small.tile([128, 2], f32)
            nc.vector.memset(cst128[:, 0:1], 0.0)
            nc.vector.memset(cst128[:, 1:2], BIAS_B)
            cst4 = small.tile([NSTEP, 2], f32)
            nc.vector.memset(cst4[:, 0:1], 0.0)
            nc.vector.memset(cst4[:, 1:2], BIAS_B)
            cst1 = small.tile([1, 1], f32)
            nc.vector.memset(cst1[:], -10.0)

            # ---- Exponentials (mask-matmul operands first) ----
            arm_e = small.tile([128, 2, NSTEP], f32r)
            nc.scalar.activation(arm_e[:], armn_sb[:], Exp,
                                 bias=cst128[:, 0:1], scale=SCALE)
            # Block-diagonal conv lhsT (host fills off-diag with -200 -> 0)
            at_t = small.tile([128, 8, NSTEP], f32r)
            nc.scalar.activation(at_t[:], atd_sb[:], Exp,
                                 bias=cst128[:, 0:1], scale=SCALE)
            sums = small.tile([NSTEP, 2], f32)  # (sum alpha, sum beta)
            bpe = small.tile([NSTEP, D], f32)
            nc.scalar.activation(bpe[:], bpap_sb[:, 0:D], Exp,
                                 bias=cst4[:, 1:2], scale=SCALE)
            ape = small.tile([NSTEP, D], f32)
            nc.scalar.activation(ape[:], bpap_sb[:, D : 2 * D], Exp,
                                 bias=cst4[:, 0:1], scale=SCALE)
            cbe = big.tile([128, 8, D], f32r)
            nc.scalar.activation(cbe[:, 0:4, :], cb_sb[:, 0:4, :], Exp,
                                 bias=cst128[:, 1:2], scale=SCALE)
            nc.scalar.activation(cbe[:, 4:8, :], cb_sb[:, 4:8, :], Exp,
                                 bias=cst128[:, 1:2], scale=SCALE)
            nc.vector.tensor_reduce(out=sums[:, 0:1], in_=ape[:],
                                    axis=mybir.AxisListType.X,
                                    op=mybir.AluOpType.add)
            nc.vector.tensor_reduce(out=sums[:, 1:2], in_=bpe[:],
                                    axis=mybir.AxisListType.X,
                                    op=mybir.AluOpType.add)

            # ---- Matmuls: masks first (they head the long scalar-chain
            # dependency path), conv T after ----
            pm = ps.tile([NSTEP, 2 * D], f32, tag="pm")
            for h in range(2):
                nc.tensor.matmul(pm[:],
                                 lhsT=arm_e[:, h, :],
                                 rhs=m12_sb[:, h, :],
                                 start=(h == 0), stop=(h == 1))
            pc = ps.tile([NSTEP, D], f32, tag="pc")
            for c in range(8):
                nc.tensor.matmul(pc[:], lhsT=at_t[:, c, :],
                                 rhs=cbe[:, c, :],
                                 start=(c == 0), stop=(c == 7))

            # ---- Scalars: Z, U1, U1' -> q1, q2 ----
            zz = small.tile([NSTEP, 2], f32)  # (Z, 1/Z)
            nc.vector.tensor_tensor(out=zz[:, 0:1], in0=sums[:, 0:1],
                                    in1=sums[:, 1:2], op=mult)
            nc.vector.reciprocal(zz[:, 1:2], zz[:, 0:1])
            uu = small.tile([NSTEP, 2], f32)  # (U1, U1')
            scr = small.tile([NSTEP, D], f32)
            nc.vector.tensor_tensor(out=scr[:], in0=pm[:, 0:D], in1=bpe[:],
                                    op=mult)
            nc.vector.tensor_reduce(out=uu[:, 0:1], in_=scr[:],
                                    axis=mybir.AxisListType.X,
                                    op=mybir.AluOpType.add)
            nc.vector.tensor_tensor(out=scr[:], in0=pm[:, D : 2 * D],
                                    in1=bpe[:], op=mult)
            nc.vector.tensor_reduce(out=uu[:, 1:2], in_=scr[:],
                                    axis=mybir.AxisListType.X,
                                    op=mybir.AluOpType.add)
            chn = small.tile([NSTEP, 2], f32)  # (q1, q2) = (U1/Z, (U1'-U1)/Z)
            nc.vector.tensor_tensor(out=chn[:, 0:1], in0=uu[:, 0:1],
                                    in1=zz[:, 1:2], op=mult)
            nc.vector.tensor_tensor(out=chn[:, 1:2], in0=uu[:, 1:2],
                                    in1=uu[:, 0:1], op=subtract)
            nc.vector.tensor_tensor(out=chn[:, 1:2], in0=chn[:, 1:2],
                                    in1=zz[:, 1:2], op=mult)

            # ---- Repartition (q1,q2) to a partition-0 row: SBUF->SBUF DMA ----
            ptc12 = small.tile([1, NSTEP, 2], f32)
            nc.sync.dma_start(ptc12[:], chn[:])

            # ---- Sequential carry chain on [1,1] scalars ----
            pcur = small.tile([1, 1], f32)
            nc.vector.memset(pcur[:], 0.0)
            lam = small.tile([1, NSTEP], f32)
            for s in range(NSTEP):
                nc.scalar.activation(lam[:, s : s + 1], pcur[:], Sigmoid,
                                     bias=cst1[:], scale=20.0)
                if s + 1 < NSTEP:
                    nc.vector.scalar_tensor_tensor(
                        out=pcur[:], in0=lam[:, s : s + 1],
                        scalar=ptc12[:, s, 1:2], in1=ptc12[:, s, 0:1],
                        op0=mult, op1=add,
                    )

            # ---- lam back to a [4,1] column: SBUF->SBUF DMA ----
            lamc = small.tile([NSTEP, 1], f32)
            nc.sync.dma_start(lamc[:], lam[:])

            # ---- Final blend: out = (T + lam*(rot1(T)-T)) / Z ----
            tsb = small.tile([NSTEP, D], f32)
            nc.vector.tensor_copy(out=tsb[:], in_=pc[:])
            trot = small.tile([NSTEP, D], f32)
            nc.vector.tensor_copy(out=trot[:, 1:D], in_=tsb[:, 0 : D - 1])
            nc.vector.tensor_copy(out=trot[:, 0:1], in_=tsb[:, D - 1 : D])
            nc.vector.tensor_tensor(out=trot[:], in0=trot[:], in1=tsb[:],
                                    op=subtract)
            res = small.tile([NSTEP, D], f32)
            nc.vector.tensor_scalar(out=res[:], in0=trot[:],
                                    scalar1=lamc[:, 0:1], scalar2=None,
                                    op0=mult)
            nc.vector.tensor_tensor(out=res[:], in0=res[:], in1=tsb[:], op=add)
            nc.vector.tensor_scalar(out=res[:], in0=res[:],
                                    scalar1=zz[:, 1:2], scalar2=None, op0=mult)
            nc.sync.dma_start(out[:], res[:])

    nc.compile()
    return nc


def _structure_ok(W1, W2_sum, W2_carry):
    """Exact check that the tables are the canonical one-hot construction."""
    if W1.shape != (514, NE) or W2_sum.shape != (NE, D) or W2_carry.shape != (NE, 2):
        return False
    k = np.arange(NE)
    a = k >> 9
    b = (k >> 1) & 255
    c = k & 1
    tot = a + b + c
    if not (W1[a, k] == 1.0).all():
        return False
    if not (W1[D + b, k] == 1.0).all():
        return False
    if not (W1[2 * D + c, k] == 1.0).all():
        return False
    if W1.sum(dtype=np.float64) != 3.0 * NE or W1.min() < 0.0:
        return False
    if not (W2_sum[k, tot & 255] == 1.0).all():
        return False
    if W2_sum.sum(dtype=np.float64) != NE or W2_sum.min() < 0.0:
        return False
    if not (W2_carry[k, (tot >= 256).astype(np.int64)] == 1.0).all():
        return False
    if W2_carry.sum(dtype=np.float64) != NE or W2_carry.min() < 0.0:
        return False
    return True


def _numpy_fallback(a_emb, b_emb, W1, W2_sum, W2_carry):
    carry = np.zeros(2, dtype=np.float64)
    carry[0] = 1.0
    outs = []
    W1 = W1.astype(np.float64)
    for i in range(NSTEP):
        x = np.concatenate([a_emb[i], b_emb[i], carry]).astype(np.float64)
        scores = x @ W1
        z = (scores - 2.5) * 10.0
        z -= z.max()
        w = np.exp(z)
        w /= w.sum()
        outs.append(w @ W2_sum.astype(np.float64))
        carry = w @ W2_carry.astype(np.float64)
    return np.stack(outs).astype(np.float32)


def _prep_inputs(a_emb, b_emb):
    """Pure-layout host gathers; every core gets the identical map."""
    idx = np.arange(D)
    # cb[p, 2s+h, m] = b_emb[s, (m + 128h + p) & 255]
    p = np.arange(128)
    cb = np.empty((128, 8, D), np.float32)
    atd = np.full((128, 8, NSTEP), -200.0, np.float32)
    armn = np.empty((128, 2, NSTEP), np.float32)
    for s in range(NSTEP):
        for h in range(2):
            aprime = 128 * h + p  # [128]
            cb[:, 2 * s + h, :] = b_emb[s][(idx[None, :] + aprime[:, None]) & 255]
            vals = a_emb[s][(256 - aprime) & 255]
            atd[:, 2 * s + h, s] = vals
            armn[:, h, s] = vals

    # m12[p, h, j] = M1[128h+p, j]; m12[p, h, 256+j] = M2[128h+p, j]
    aprime = idx[:, None]  # [256,1]
    M1 = ((idx[None, :] >= aprime) & (aprime >= 1)).astype(np.float32)
    M2 = ((idx[None, :] >= aprime - 1) & (aprime >= 1)).astype(np.float32)
    M2[0, :] = 0.0
    M2[0, 255] = 1.0
    m12 = np.empty((128, 2, 2 * D), np.float32)
    for h in range(2):
        m12[:, h, 0:D] = M1[128 * h : 128 * h + 128, :]
        m12[:, h, D : 2 * D] = M2[128 * h : 128 * h + 128, :]

    bpap = np.empty((NSTEP, 2 * D), np.float32)
    bpap[:, 0:D] = b_emb
    bpap[:, D : 2 * D] = a_emb
    one = {
        "cb": cb,
        "m12": m12,
        "atd": atd,
        "armn": armn,
        "bpap": bpap,
    }
    return [dict(one) for _ in range(N_CORES)]


def kernel(a_emb, b_emb, W1, W2_sum, W2_carry):
    global LAST_EXEC_TIME_NS
    a_emb = np.asarray(a_emb, dtype=np.float32)
    b_emb = np.asarray(b_emb, dtype=np.float32)
    W1 = np.asarray(W1, dtype=np.float32)
    W2_sum = np.asarray(W2_sum, dtype=np.float32)
    W2_carry = np.asarray(W2_carry, dtype=np.float32)

    if not _structure_ok(W1, W2_sum, W2_carry):
        return _numpy_fallback(a_emb, b_emb, W1, W2_sum, W2_carry)

    from concourse.bass_utils import run_bass_kernel_spmd

    if "nc" not in _CACHE:
        _CACHE["nc"] = _build_nc()
    nc = _CACHE["nc"]

    in_maps = _prep_inputs(a_emb, b_emb)
    trace = os.environ.get("KERNEL_TRACE", "") == "1"
    res = run_bass_kernel_spmd(nc, in_maps, list(range(N_CORES)), trace=trace)
    LAST_EXEC_TIME_NS = res.exec_time_ns
    return np.asarray(res.results[0]["out"], dtype=np.float32)


# revision 30
# speedup vs baseline: 1.3191x; 1.0629x over previous
"""Trainium2 Bass kernel for nn_C4ByteTransformer (4-step carry-propagation
softmax table lookup).

Contract: kernel(**inputs) takes FULL inputs (a_emb[4,256], b_emb[4,256],
W1[514,131072], W2_sum[131072,256], W2_carry[131072,2]) and returns the full
[4,256] float32 output.

Math: the tables are the canonical one-hot construction (verified exactly on
host, numpy fallback otherwise), so
  scores[k] = a_i[a(k)] + b_i[b(k)] + carry[c(k)],  k = 512a + 2b + c.
The softmax weights therefore factorize rank-1 per carry slice:
  w[a,b,c] ~ alpha[a] * beta[b] * gamma_c,
  alpha = exp(10*a_i), beta = exp(10*b_i - 25), gamma_c = exp(10*carry_c),
and the entire step reduces to a length-256 circular convolution
  T[m]   = sum_a alpha[a] * beta[(m-a) & 255]          (result numerator)
plus two triangular sums for the carry:
  U1     = sum_{a+b>=256} alpha[a]*beta[b]
  U1'    = sum_{a+b>=255} alpha[a]*beta[b]
  Z      = sum(alpha) * sum(beta)
With p = P(carry=1), lam = sigmoid(10*(2p-1)):
  out_s  = (T + lam*(rot1(T) - T)) / Z
  p_next = (U1 + lam*(U1' - U1)) / Z
No table reads, no collectives: each core runs the identical tiny program
(~1.5MB of host-gathered circulant/mask operands) and core 0's output is
returned.  The convolution is one 8-chunk f32 matmul against a host-gathered
circulant of b_emb; the triangular sums are one 2-chunk matmul against
constant 0/1 masks; the 4-step carry chain runs on [1,1] scalars.
"""

import os

import numpy as np

N_CORES = 8
D = 256
NSTEP = 4
NE = 256 * 256 * 2
SCALE = 10.0
BIAS_B = -25.0

_CACHE = {}

LAST_EXEC_TIME_NS = None


def _build_nc():
    import concourse.bacc as bacc
    import concourse.mybir as mybir
    import concourse.tile as tile

    f32 = mybir.dt.float32
    f32r = mybir.dt.float32r
    mult = mybir.AluOpType.mult
    add = mybir.AluOpType.add
    subtract = mybir.AluOpType.subtract
    Exp = mybir.ActivationFunctionType.Exp
    Sigmoid = mybir.ActivationFunctionType.Sigmoid

    nc = bacc.Bacc("TRN2", target_bir_lowering=False, debug=False,
                   num_devices=N_CORES)

    # Inputs (host pre-gathered; identical on every core).
    cb = nc.dram_tensor("cb", [128, 8, D], f32, kind="ExternalInput")
    m12 = nc.dram_tensor("m12", [128, 2, 2 * D], f32r, kind="ExternalInput")
    atd = nc.dram_tensor("atd", [128, 8, NSTEP], f32, kind="ExternalInput")
    armn = nc.dram_tensor("armn", [128, 2, NSTEP], f32, kind="ExternalInput")
    bpap = nc.dram_tensor("bpap", [NSTEP, 2 * D], f32, kind="ExternalInput")
    out = nc.dram_tensor("out", [NSTEP, D], f32, kind="ExternalOutput")

    with tile.TileContext(nc) as tc:
        with (
            tc.tile_pool(name="big", bufs=1) as big,
            tc.tile_pool(name="small", bufs=1) as small,
            tc.tile_pool(name="ps", bufs=1, space="PSUM") as ps,
        ):
            # ---- DMAs: packed smalls first, big circulant split across the
            # two HWDGE queues, masks last (needed ~mid-kernel) ----
            m12_sb = big.tile([128, 2, 2 * D], f32r)
            nc.sync.dma_start(m12_sb[:], m12[:])
            armn_sb = small.tile([128, 2, NSTEP], f32)
            nc.scalar.dma_start(armn_sb[:], armn[:])
            bpap_sb = small.tile([NSTEP, 2 * D], f32)
            nc.scalar.dma_start(bpap_sb[:], bpap[:])
            atd_sb = small.tile([128, 8, NSTEP], f32)
            nc.sync.dma_start(atd_sb[:], atd[:])
            cb_sb = big.tile([128, 8, D], f32)
            nc.sync.dma_start(cb_sb[:, 0:4, :], cb[:, 0:4, :])
            nc.scalar.dma_start(cb_sb[:, 4:8, :], cb[:, 4:8, :])

            # Constant bias tiles for ACT (float biases need const APs).
            cst128 = small.tile([128, 2], f32)
            nc.vector.memset(cst128[:, 0:1], 0.0)
            nc.vector.memset(cst128[:, 1:2], BIAS_B)
            cst4 = small.tile([NSTEP, 2], f32)
            nc.vector.memset(cst4[:, 0:1], 0.0)
            nc.vector.memset(cst4[:, 1:2], BIAS_B)
            cst1 = small.tile([1, 1], f32)
            nc.vector.memset(cst1[:], 10.0)

            # ---- Exponentials (mask-matmul operands first) ----
            arm_e = small.tile([128, 2, NSTEP], f32r)
            nc.scalar.activation(arm_e[:], armn_sb[:], Exp,
                                 bias=cst128[:, 0:1], scale=SCALE)
            # Block-diagonal conv lhsT (host fills off-diag with -200 -> 0)
            at_t = small.tile([128, 8, NSTEP], f32r)
            nc.scalar.activation(at_t[:], atd_sb[:], Exp,
                                 bias=cst128[:, 0:1], scale=SCALE)
            sums = small.tile([NSTEP, 2], f32)  # (sum alpha, sum beta)
            bpe = small.tile([NSTEP, D], f32)
            nc.scalar.activation(bpe[:], bpap_sb[:, 0:D], Exp,
                                 bias=cst4[:, 1:2], scale=SCALE)
            ape = small.tile([NSTEP, D], f32)
            nc.scalar.activation(ape[:], bpap_sb[:, D : 2 * D], Exp,
                                 bias=cst4[:, 0:1], scale=SCALE)
            cbe = big.tile([128, 8, D], f32r)
            nc.scalar.activation(cbe[:, 0:4, :], cb_sb[:, 0:4, :], Exp,
                                 bias=cst128[:, 1:2], scale=SCALE)
            nc.scalar.activation(cbe[:, 4:8, :], cb_sb[:, 4:8, :], Exp,
                                 bias=cst128[:, 1:2], scale=SCALE)
            nc.vector.tensor_reduce(out=sums[:, 0:1], in_=ape[:],
                                    axis=mybir.AxisListType.X,
                                    op=mybir.AluOpType.add)
            nc.vector.tensor_reduce(out=sums[:, 1:2], in_=bpe[:],
                                    axis=mybir.AxisListType.X,
                                    op=mybir.AluOpType.add)

            # ---- Matmuls: masks first (they head the long scalar-chain
            # dependency path), conv T after ----
            pm = ps.tile([NSTEP, 2 * D], f32, tag="pm")
            for h in range(2):
                nc.tensor.matmul(pm[:],
                                 lhsT=arm_e[:, h, :],
                                 rhs=m12_sb[:, h, :],
                                 start=(h == 0), stop=(h == 1))
            pc = ps.tile([NSTEP, D], f32, tag="pc")
            for c in range(8):
                nc.tensor.matmul(pc[:], lhsT=at_t[:, c, :],
                                 rhs=cbe[:, c, :],
                                 start=(c == 0), stop=(c == 7))

            # ---- Scalars: Z, U1, U1' -> q1, q2 ----
            zz = small.tile([NSTEP, 2], f32)  # (Z, 1/Z)
            nc.vector.tensor_tensor(out=zz[:, 0:1], in0=sums[:, 0:1],
                                    in1=sums[:, 1:2], op=mult)
            nc.vector.reciprocal(zz[:, 1:2], zz[:, 0:1])
            uu = small.tile([NSTEP, 2], f32)  # (U1, U1')
            scr = small.tile([NSTEP, D], f32)
            nc.vector.tensor_tensor(out=scr[:], in0=pm[:, 0:D], in1=bpe[:],
                                    op=mult)
            nc.vector.tensor_reduce(out=uu[:, 0:1], in_=scr[:],
                                    axis=mybir.AxisListType.X,
                                    op=mybir.AluOpType.add)
            nc.vector.tensor_tensor(out=scr[:], in0=pm[:, D : 2 * D],
                                    in1=bpe[:], op=mult)
            nc.vector.tensor_reduce(out=uu[:, 1:2], in_=scr[:],
                                    axis=mybir.AxisListType.X,
                                    op=mybir.AluOpType.add)
            chn = small.tile([NSTEP, 2], f32)  # (q1, q2) = (U1/Z, (U1'-U1)/Z)
            nc.vector.tensor_tensor(out=chn[:, 0:1], in0=uu[:, 0:1],
                                    in1=zz[:, 1:2], op=mult)
            nc.vector.tensor_tensor(out=chn[:, 1:2], in0=uu[:, 1:2],
                                    in1=uu[:, 0:1], op=subtract)
            nc.vector.tensor_tensor(out=chn[:, 1:2], in0=chn[:, 1:2],
                                    in1=zz[:, 1:2], op=mult)

            # ---- Repartition (q1,q2) to a partition-0 row: SBUF->SBUF DMA ----
            ptc12 = small.tile([1, NSTEP, 2], f32)
            nc.sync.dma_start(ptc12[:], chn[:])

            # ---- Sequential carry chain on [1,1] scalars.
            # e_s = exp(10-20p); lam_s = 1/(1+e_s); p' = q1 + lam*q2.
            # Exp only (no Sigmoid): avoids a second ACT table load.
            pcur = small.tile([1, 1], f32)
            nc.vector.memset(pcur[:], 0.0)
            erow = small.tile([1, NSTEP], f32)
            wtmp = small.tile([1, 2], f32)
            for s in range(NSTEP):
                nc.scalar.activation(erow[:, s : s + 1], pcur[:], Exp,
                                     bias=cst1[:], scale=-20.0)
                if s + 1 < NSTEP:
                    nc.vector.tensor_scalar(out=wtmp[:, 0:1],
                                            in0=erow[:, s : s + 1],
                                            scalar1=1.0, scalar2=None, op0=add)
                    nc.vector.reciprocal(wtmp[:, 1:2], wtmp[:, 0:1])
                    nc.vector.scalar_tensor_tensor(
                        out=pcur[:], in0=ptc12[:, s, 1:2],
                        scalar=wtmp[:, 1:2], in1=ptc12[:, s, 0:1],
                        op0=mult, op1=add,
                    )

            # ---- e back to a [4,1] column; lam = 1/(1+e) vectorized ----
            ecol = small.tile([NSTEP, 1], f32)
            nc.sync.dma_start(ecol[:], erow[:])
            lamc = small.tile([NSTEP, 2], f32)
            nc.vector.tensor_scalar(out=lamc[:, 0:1], in0=ecol[:],
                                    scalar1=1.0, scalar2=None, op0=add)
            nc.vector.reciprocal(lamc[:, 1:2], lamc[:, 0:1])

            # ---- Final blend: out = (T + lam*(rot1(T)-T)) / Z ----
            tsb = small.tile([NSTEP, D], f32)
            nc.vector.tensor_copy(out=tsb[:], in_=pc[:])
            trot = small.tile([NSTEP, D], f32)
            nc.vector.tensor_copy(out=trot[:, 1:D], in_=tsb[:, 0 : D - 1])
            nc.vector.tensor_copy(out=trot[:, 0:1], in_=tsb[:, D - 1 : D])
            nc.vector.tensor_tensor(out=trot[:], in0=trot[:], in1=tsb[:],
                                    op=subtract)
            res = small.tile([NSTEP, D], f32)
            nc.vector.tensor_scalar(out=res[:], in0=trot[:],
                                    scalar1=lamc[:, 1:2], scalar2=None,
                                    op0=mult)
            nc.vector.tensor_tensor(out=res[:], in0=res[:], in1=tsb[:], op=add)
            nc.vector.tensor_scalar(out=res[:], in0=res[:],
                                    scalar1=zz[:, 1:2], scalar2=None, op0=mult)
            nc.sync.dma_start(out[:], res[:])

    nc.compile()
    return nc


def _structure_ok(W1, W2_sum, W2_carry):
    """Exact check that the tables are the canonical one-hot construction."""
    if W1.shape != (514, NE) or W2_sum.shape != (NE, D) or W2_carry.shape != (NE, 2):
        return False
    k = np.arange(NE)
    a = k >> 9
    b = (k >> 1) & 255
    c = k & 1
    tot = a + b + c
    if not (W1[a, k] == 1.0).all():
        return False
    if not (W1[D + b, k] == 1.0).all():
        return False
    if not (W1[2 * D + c, k] == 1.0).all():
        return False
    if W1.sum(dtype=np.float64) != 3.0 * NE or W1.min() < 0.0:
        return False
    if not (W2_sum[k, tot & 255] == 1.0).all():
        return False
    if W2_sum.sum(dtype=np.float64) != NE or W2_sum.min() < 0.0:
        return False
    if not (W2_carry[k, (tot >= 256).astype(np.int64)] == 1.0).all():
        return False
    if W2_carry.sum(dtype=np.float64) != NE or W2_carry.min() < 0.0:
        return False
    return True


def _numpy_fallback(a_emb, b_emb, W1, W2_sum, W2_carry):
    carry = np.zeros(2, dtype=np.float64)
    carry[0] = 1.0
    outs = []
    W1 = W1.astype(np.float64)
    for i in range(NSTEP):
        x = np.concatenate([a_emb[i], b_emb[i], carry]).astype(np.float64)
        scores = x @ W1
        z = (scores - 2.5) * 10.0
        z -= z.max()
        w = np.exp(z)
        w /= w.sum()
        outs.append(w @ W2_sum.astype(np.float64))
        carry = w @ W2_carry.astype(np.float64)
    return np.stack(outs).astype(np.float32)


def _prep_inputs(a_emb, b_emb):
    """Pure-layout host gathers; every core gets the identical map."""
    idx = np.arange(D)
    # cb[p, 2s+h, m] = b_emb[s, (m + 128h + p) & 255]
    p = np.arange(128)
    cb = np.empty((128, 8, D), np.float32)
    atd = np.full((128, 8, NSTEP), -200.0, np.float32)
    armn = np.empty((128, 2, NSTEP), np.float32)
    for s in range(NSTEP):
        for h in range(2):
            aprime = 128 * h + p  # [128]
            cb[:, 2 * s + h, :] = b_emb[s][(idx[None, :] + aprime[:, None]) & 255]
            vals = a_emb[s][(256 - aprime) & 255]
            atd[:, 2 * s + h, s] = vals
            armn[:, h, s] = vals

    # m12[p, h, j] = M1[128h+p, j]; m12[p, h, 256+j] = M2[128h+p, j]
    aprime = idx[:, None]  # [256,1]
    M1 = ((idx[None, :] >= aprime) & (aprime >= 1)).astype(np.float32)
    M2 = ((idx[None, :] >= aprime - 1) & (aprime >= 1)).astype(np.float32)
    M2[0, :] = 0.0
    M2[0, 255] = 1.0
    m12 = np.empty((128, 2, 2 * D), np.float32)
    for h in range(2):
        m12[:, h, 0:D] = M1[128 * h : 128 * h + 128, :]
        m12[:, h, D : 2 * D] = M2[128 * h : 128 * h + 128, :]

    bpap = np.empty((NSTEP, 2 * D), np.float32)
    bpap[:, 0:D] = b_emb
    bpap[:, D : 2 * D] = a_emb
    one = {
        "cb": cb,
        "m12": m12,
        "atd": atd,
        "armn": armn,
        "bpap": bpap,
    }
    return [dict(one) for _ in range(N_CORES)]


def kernel(a_emb, b_emb, W1, W2_sum, W2_carry):
    global LAST_EXEC_TIME_NS
    a_emb = np.asarray(a_emb, dtype=np.float32)
    b_emb = np.asarray(b_emb, dtype=np.float32)
    W1 = np.asarray(W1, dtype=np.float32)
    W2_sum = np.asarray(W2_sum, dtype=np.float32)
    W2_carry = np.asarray(W2_carry, dtype=np.float32)

    if not _structure_ok(W1, W2_sum, W2_carry):
        return _numpy_fallback(a_emb, b_emb, W1, W2_sum, W2_carry)

    from concourse.bass_utils import run_bass_kernel_spmd

    if "nc" not in _CACHE:
        _CACHE["nc"] = _build_nc()
    nc = _CACHE["nc"]

    in_maps = _prep_inputs(a_emb, b_emb)
    trace = os.environ.get("KERNEL_TRACE", "") == "1"
    res = run_bass_kernel_spmd(nc, in_maps, list(range(N_CORES)), trace=trace)
    LAST_EXEC_TIME_NS = res.exec_time_ns
    return np.asarray(res.results[0]["out"], dtype=np.float32)
